# revision 8
# baseline (speedup 1.0000x reference)
"""GAT (graph attention) kernel for Trainium2, 8-core SPMD.

Per core (dst-sharded), v4:
  The source-node table (xw = x @ W, 256B bf16 rows) is stored in
  BLOCK-MAJOR permuted order: 6-tile blocks, rows (block, p, tile) - so
  each phase-1 block write is one contiguous ~1.5KB descriptor per
  partition, AND each of the 4 int16 gather windows is a contiguous
  range of blocks held in its OWN dram tensor.  Gathers for window w
  therefore depend only on window w's table writes: the phase-2 gather
  stream starts ~75us in (after the first window is built), overlapping
  the rest of phase 1.  The gather stream is the kernel's critical
  resource (~2.7ns/row, 4 SWDGE queues); everything else hides under it.

  Phase 1b: own-shard pass computes out_partial[d] = ee_loop[d]*xw[d]+bias
  into SBUF bf16 (rank-1 bias matmul + ACT per-partition-scale copy).

  Phase 2 runs WINDOW-MAJOR: for each window w, for each group of `gsz`
  dst tiles: one dma_gather segment (per-tile slot ranges packed
  contiguously - only the segment rounds to 128 slots; boundary chunks
  get one routing matmul per tile, a "role").  A one-hot-times-ee matrix
  M[slot, dst] per role (DVE is_equal+mult build, or HBM upload for
  UPLOAD_NUM/UPLOAD_DEN of groups - uploads ride the idle HWDGE queues)
  routes edges: psum_t += M^T @ G; after each (tile, window):
  acc_t (+)= psum (bf16 SBUF accumulator).  Final: out = relu(acc +
  out_partial), written p-major (host un-permutes).

Host precomputes per-edge normalized attention coefficients
(two matvecs + O(E) scalar math, ~0.2% of total FLOPs) and the edge->slot
schedule; the feature matmul, the edge gather, aggregation matmuls and the
nonlinearity run on device.  Softmax max-subtraction is unnecessary:
a_s, a_d ~ N(0,1) so logits stay O(10) and exp() is safe in fp32.
Padding slots get ee = 0 so they contribute nothing.
"""

import numpy as np
import ml_dtypes

BF16 = ml_dtypes.bfloat16

# problem constants (nn_GAT_43593918054566)
N_NODES = 100000
F_IN = 256
HID = 128
NEG_SLOPE = 0.2
N_CORES = 8
UPLOAD_NUM, UPLOAD_DEN = 1, 2   # upload M for NUM of every DEN groups
BT = 6                          # table tiles per phase-1 block


class Geo:
    """Geometry/schedule shared by host prep and kernel builder."""

    def __init__(self, n_nodes=N_NODES, f_in=F_IN, hid=HID, n_cores=N_CORES,
                 sh_tiles=98, group_tiles=5):
        self.n = n_nodes
        self.f_in = f_in
        self.hid = hid
        self.n_cores = n_cores
        self.ntiles_tab = -(-n_nodes // 128)          # node tiles in table
        self.ntab = self.ntiles_tab * 128             # padded table rows
        self.sh_tiles = sh_tiles                      # dst tiles per core
        self.sh = sh_tiles * 128                      # dst shard stride
        assert self.sh * (n_cores - 1) < n_nodes <= self.sh * n_cores
        # phase-1 blocks of BT tiles; 4 gather windows = block ranges
        self.nblk = -(-self.ntiles_tab // BT)
        self.blk_nt = [min(BT, self.ntiles_tab - b * BT)
                       for b in range(self.nblk)]
        self.blk_row0 = np.concatenate(
            [[0], np.cumsum([nt * 128 for nt in self.blk_nt])]).astype(np.int64)
        nw = 4
        bw = -(-self.nblk // nw)
        self.wblk = [min(i * bw, self.nblk) for i in range(nw + 1)]
        self.wrow0 = [int(self.blk_row0[self.wblk[i]]) for i in range(nw + 1)]
        for i in range(nw):
            assert self.wrow0[i + 1] - self.wrow0[i] <= 32768
        self.gsz = group_tiles                        # dst tiles per group
        self.ng = -(-sh_tiles // group_tiles)

    def core_dst_range(self, c):
        lo = self.sh * c
        hi = min(lo + self.sh, self.n)
        return lo, hi

    def perm_row(self, n):
        """HBM table row of node n (block-major permuted layout)."""
        n = np.asarray(n)
        t = n >> 7
        p = n & 127
        b = t // BT
        a = t - b * BT
        nt = np.minimum(BT, self.ntiles_tab - b * BT)
        return self.blk_row0[b] + p * nt + a


def _prep(geo, x, edge_index, W, att_src, att_dst, bias):
    """Host preprocessing: edge partitioning + per-core input arrays."""
    g = geo
    x = np.asarray(x, dtype=np.float32)
    W = np.asarray(W, dtype=np.float32)
    esrc = np.asarray(edge_index[0], dtype=np.int64)
    edst = np.asarray(edge_index[1], dtype=np.int64)

    # per-edge normalized attention (host: 2 matvecs + O(E) scalar math)
    a_s = x @ (W @ np.asarray(att_src, np.float32))
    a_d = x @ (W @ np.asarray(att_dst, np.float32))

    def ee_of(s, d):
        e = a_s[s] + a_d[d]
        e = np.where(e > 0, e, NEG_SLOPE * e)
        return np.exp(e).astype(np.float32)

    ee_reg_all = ee_of(esrc, edst)
    ee_loop = ee_of(np.arange(g.n), np.arange(g.n))   # self loops
    denom = ee_loop.astype(np.float64).copy()
    np.add.at(denom, edst, ee_reg_all.astype(np.float64))
    ee_reg_all = (ee_reg_all / denom[edst]).astype(np.float32)
    ee_loop = (ee_loop / denom).astype(np.float32)

    # permuted gather rows + window of every edge source
    rperm_all = g.perm_row(esrc)
    wbs = np.asarray(g.wrow0[1:], dtype=np.int64)

    cores = []
    for c in range(g.n_cores):
        lo, hi = g.core_dst_range(c)
        m = (edst >= lo) & (edst < hi)
        d_c = edst[m] - lo
        t_c = d_c >> 7
        rp_c = rperm_all[m]
        r_c = np.searchsorted(wbs, rp_c, side="right")
        cores.append((rp_c, d_c, t_c, r_c, ee_reg_all[m]))

    # per-(tile, window) slot quota: max edge count over cores (NOT rounded)
    counts = np.zeros((g.n_cores, g.sh_tiles, 4), dtype=np.int64)
    for c, (_, _, t_c, r_c, _) in enumerate(cores):
        np.add.at(counts[c], (t_c, r_c), 1)
    Q = counts.max(axis=0)  # [T, 4]

    # slot layout: segment = (window, group); tiles packed contiguously,
    # segment rounded to 128.  Emission order is window-major: (r, gi).
    tile_slot0 = np.zeros((g.sh_tiles, 4), dtype=np.int64)
    segs = {}              # (r, gi) -> (first_chunk, n_chunks)
    roles = []             # (chunk, tile, gi, r) in emission order
    tile_roles = {}        # (t, r) -> [role_id...]
    groups = [list(range(gi * g.gsz, min((gi + 1) * g.gsz, g.sh_tiles)))
              for gi in range(g.ng)]
    off = 0
    for r in range(4):
        for gi in range(g.ng):
            tiles = groups[gi]
            seg_first_ch = off // 128
            so = off
            for t in tiles:
                tile_slot0[t, r] = so
                so += int(Q[t, r])
            so = -(-so // 128) * 128
            nchk = (so - off) // 128
            if nchk > 0:
                segs[(r, gi)] = (seg_first_ch, nchk)
            for t in tiles:
                a, b = tile_slot0[t, r], tile_slot0[t, r] + Q[t, r]
                if b == a:
                    continue
                for k in range(int(a // 128), int(-(-b // 128))):
                    rid = len(roles)
                    roles.append((k, t, gi, r))
                    tile_roles.setdefault((t, r), []).append(rid)
            off = so
    nch = off // 128
    nslot = off
    nroles = len(roles)

    up_groups = set(gi for gi in range(g.ng)
                    if (gi * UPLOAD_NUM) % UPLOAD_DEN < UPLOAD_NUM)
    up_role_off = {}
    bld_role_off = {}
    nup = nbld = 0
    for rid, (k, t, gi, r) in enumerate(roles):
        if gi in up_groups:
            up_role_off[rid] = nup
            nup += 1
        else:
            bld_role_off[rid] = nbld
            nbld += 1
    nup_a = max(nup, 1)
    nbld_a = max(nbld, 1)

    # slot -> tile map (core-uniform)
    slot_tile = np.full(nslot, -1, dtype=np.int64)
    for t in range(g.sh_tiles):
        for r in range(4):
            a = tile_slot0[t, r]
            slot_tile[a:a + Q[t, r]] = t

    per_core = []
    for c, (rp_c, d_c, t_c, r_c, ee_c) in enumerate(cores):
        lo, hi = g.core_dst_range(c)
        idx_flat = np.zeros(nslot, dtype=np.int16)
        dmod = np.zeros(nslot, dtype=np.int32)
        eesl = np.zeros(nslot, dtype=np.float32)
        order = np.lexsort((t_c, r_c))
        rp_o, d_o, t_o, r_o = rp_c[order], d_c[order], t_c[order], r_c[order]
        ee_o = ee_c[order]
        run_id = r_o * g.sh_tiles + t_o
        run_starts = np.searchsorted(run_id, np.arange(4 * g.sh_tiles))
        rank = np.arange(len(rp_o)) - run_starts[run_id]
        slot = tile_slot0[t_o, r_o] + rank
        idx_flat[slot] = (rp_o - np.asarray(g.wrow0, dtype=np.int64)[r_o]).astype(np.int16)
        dmod[slot] = (d_o & 127).astype(np.int32)
        eesl[slot] = ee_o

        # wrap gather idx per segment: pos i -> [16k + i%16, i//16]
        idx16 = np.zeros((128, nslot // 16), dtype=np.int16)
        for (r, gi), (seg_first, seg_nch) in segs.items():
            a, b = seg_first * 128, (seg_first + seg_nch) * 128
            wrapped = idx_flat[a:b].reshape(-1, 16).T
            cols = slice(a // 16, b // 16)
            for k in range(8):
                idx16[16 * k:16 * k + 16, cols] = wrapped
        # per-role dmod/ee columns (ee zero outside the role's tile range)
        dmodb = np.zeros((128, nbld_a), dtype=np.float32)
        eeb = np.zeros((128, nbld_a), dtype=np.float32)
        mh = np.zeros((nup_a, 128, 128), dtype=BF16)
        for rid, (k, t, gi, r) in enumerate(roles):
            sl = slice(k * 128, (k + 1) * 128)
            dm = dmod[sl]
            msk = slot_tile[sl] == t
            ee = np.where(msk, eesl[sl], 0.0).astype(np.float32)
            if gi in up_groups:
                mo = up_role_off[rid]
                mh[mo, np.arange(128), dm] = ee.astype(BF16)
            else:
                bo = bld_role_off[rid]
                dmodb[:, bo] = dm.astype(np.float32)
                eeb[:, bo] = ee
        mh = np.ascontiguousarray(mh.transpose(1, 0, 2))  # [128, nup, 128]
        # own-shard x (transposed, zero-padded) + rank-1 bias row 1/ee_loop
        nd = hi - lo
        xto = np.zeros((g.f_in + 1, g.sh), dtype=BF16)
        xto[:g.f_in, :nd] = x[lo:hi].T.astype(BF16)
        xto[g.f_in, :nd] = (1.0 / ee_loop[lo:hi]).astype(BF16)
        el = np.zeros(g.sh, dtype=np.float32)
        el[:nd] = ee_loop[lo:hi]
        eelp = np.ascontiguousarray(el.reshape(g.sh_tiles, 128).T)
        per_core.append({"idx": idx16, "mup": mh, "xto": xto,
                         "dmodb": dmodb, "eeb": eeb, "eeloop": eelp})

    # x transposed, node-major: the phase-1 write AP performs the
    # block-major row permutation (s[p, a] -> row lrow + p*nt + a)
    xT = np.zeros((g.f_in, g.ntab), dtype=BF16)
    xT[:, :g.n] = x.T.astype(BF16)
    wbf = np.ascontiguousarray(W.astype(BF16))
    biasr = np.asarray(bias, np.float32).astype(BF16)[None, :]
    iota128 = np.ascontiguousarray(
        np.tile(np.arange(128, dtype=np.float32).astype(BF16), (128, 1)))

    shared = {"xt": xT, "w": wbf, "biasr": biasr, "iota128": iota128}
    sched = {"nch": nch, "nslot": nslot, "nroles": nroles,
             "nup": nup_a, "nbld": nbld_a, "segs": segs, "roles": roles,
             "tile_roles": tile_roles, "groups": groups,
             "up_groups": up_groups, "up_role_off": up_role_off,
             "bld_role_off": bld_role_off}
    return shared, per_core, sched


def _build(geo, sched):
    """Build the (core-uniform) Bass program."""
    import concourse.bacc as bacc
    import concourse.mybir as mybir
    from concourse import tile
    from contextlib import ExitStack

    g = geo
    nch, nslot = sched["nch"], sched["nslot"]
    f32, bf16 = mybir.dt.float32, mybir.dt.bfloat16
    i16 = mybir.dt.int16
    Alu = mybir.AluOpType

    nc = bacc.Bacc("TRN2", target_bir_lowering=False, debug=False,
                   num_devices=g.n_cores, num_swdge_queues=4)

    xt_d = nc.dram_tensor("xt", [g.f_in, g.ntab], bf16, kind="ExternalInput")
    xto_d = nc.dram_tensor("xto", [g.f_in + 1, g.sh], bf16, kind="ExternalInput")
    w_d = nc.dram_tensor("w", [g.f_in, g.hid], bf16, kind="ExternalInput")
    biasr_d = nc.dram_tensor("biasr", [1, g.hid], bf16, kind="ExternalInput")
    idx_d = nc.dram_tensor("idx", [128, nslot // 16], i16, kind="ExternalInput")
    mup_d = nc.dram_tensor("mup", [128, sched["nup"], 128], bf16,
                           kind="ExternalInput")
    iota_d = nc.dram_tensor("iota128", [128, 128], bf16, kind="ExternalInput")
    dmodb_d = nc.dram_tensor("dmodb", [128, sched["nbld"]], f32,
                             kind="ExternalInput")
    eeb_d = nc.dram_tensor("eeb", [128, sched["nbld"]], f32,
                           kind="ExternalInput")
    eeloop_d = nc.dram_tensor("eeloop", [128, g.sh_tiles], f32,
                              kind="ExternalInput")
    out_d = nc.dram_tensor("out", [128, g.sh_tiles, g.hid], f32,
                           kind="ExternalOutput")
    # one table tensor per gather window -> exact write->gather deps
    tw_d = [nc.dram_tensor(f"tw{w}", [g.wrow0[w + 1] - g.wrow0[w], 128],
                           bf16, kind="Internal") for w in range(4)]

    with tile.TileContext(nc) as tc, ExitStack() as ctx:
        const = ctx.enter_context(tc.tile_pool(name="const", bufs=1))
        w0 = const.tile([128, g.hid], bf16)
        w1 = const.tile([128, g.hid], bf16)
        nc.sync.dma_start(w0[:], w_d[0:128, :])
        nc.sync.dma_start(w1[:], w_d[128:256, :])
        biasr_sb = const.tile([1, g.hid], bf16)
        nc.sync.dma_start(biasr_sb[:], biasr_d[:])
        idx_sb = const.tile([128, nslot // 16], i16)
        nc.sync.dma_start(idx_sb[:], idx_d[:])
        iota_sb = const.tile([128, 128], bf16)
        nc.sync.dma_start(iota_sb[:], iota_d[:])
        dmodb_sb = const.tile([128, sched["nbld"]], f32)
        nc.sync.dma_start(dmodb_sb[:], dmodb_d[:])
        eeb_sb = const.tile([128, sched["nbld"]], f32)
        nc.sync.dma_start(eeb_sb[:], eeb_d[:])
        eeloop_sb = const.tile([128, g.sh_tiles], f32)
        nc.sync.dma_start(eeloop_sb[:], eeloop_d[:])
        outp = const.tile([128, g.sh_tiles, g.hid], bf16)
        acc = const.tile([128, g.sh_tiles, g.hid], bf16)

        # ---- Phase 1a: node table, block-major windows ----
        with tc.tile_pool(name="xp", bufs=4) as xp, \
             tc.tile_pool(name="stag", bufs=3) as stag, \
             tc.tile_pool(name="ps1", bufs=7, space="PSUM") as ps1:
            for b in range(g.nblk):
                t0 = BT * b
                nt = g.blk_nt[b]
                w = next(i for i in range(4)
                         if g.wblk[i] <= b < g.wblk[i + 1])
                lrow = int(g.blk_row0[b]) - g.wrow0[w]
                xs0 = xp.tile([128, nt * 128], bf16, tag="xs0")
                xs1 = xp.tile([128, nt * 128], bf16, tag="xs1")
                c0 = int(g.blk_row0[b])
                nc.sync.dma_start(xs0[:], xt_d[0:128, c0:c0 + nt * 128])
                nc.sync.dma_start(xs1[:], xt_d[128:256, c0:c0 + nt * 128])
                s = stag.tile([128, nt, 128], bf16, tag="s")
                for h in range(-(-nt // 3)):
                    np_ = min(3, nt - 3 * h)
                    ps = ps1.tile([128, np_ * 128], f32, tag="ps1t")
                    for j in range(np_):
                        jj = 3 * h + j
                        nc.tensor.matmul(ps[:, j * 128:(j + 1) * 128],
                                         xs0[:, jj * 128:(jj + 1) * 128],
                                         w0[:], start=True, stop=False)
                        nc.tensor.matmul(ps[:, j * 128:(j + 1) * 128],
                                         xs1[:, jj * 128:(jj + 1) * 128],
                                         w1[:], start=False, stop=True)
                    nc.scalar.copy(s[:, 3 * h:3 * h + np_, :],
                                   ps[:].rearrange("p (a b) -> p a b", b=128))
                nc.scalar.dma_start(
                    tw_d[w][lrow:lrow + nt * 128, :].rearrange(
                        "(p a) e -> p a e", p=128),
                    s[:])

            # ---- Phase 1b: own shard -> out_partial ----
            for b in range(-(-g.sh_tiles // 6)):
                t0 = 6 * b
                nt = min(6, g.sh_tiles - t0)
                xs0 = xp.tile([128, nt * 128], bf16, tag="xs0")
                xs1 = xp.tile([128, nt * 128], bf16, tag="xs1")
                xs2 = xp.tile([1, nt * 128], bf16, tag="xs2")
                nc.sync.dma_start(xs0[:], xto_d[0:128, t0 * 128:(t0 + nt) * 128])
                nc.sync.dma_start(xs1[:], xto_d[128:256, t0 * 128:(t0 + nt) * 128])
                nc.sync.dma_start(xs2[:], xto_d[256:257, t0 * 128:(t0 + nt) * 128])
                for h in range(-(-nt // 3)):
                    np_ = min(3, nt - 3 * h)
                    ps = ps1.tile([128, np_ * 128], f32, tag="ps1t")
                    for j in range(np_):
                        jj = 3 * h + j
                        nc.tensor.matmul(ps[:, j * 128:(j + 1) * 128],
                                         xs0[:, jj * 128:(jj + 1) * 128],
                                         w0[:], start=True, stop=False)
                        nc.tensor.matmul(ps[:, j * 128:(j + 1) * 128],
                                         xs1[:, jj * 128:(jj + 1) * 128],
                                         w1[:], start=False, stop=False)
                        nc.tensor.matmul(ps[:, j * 128:(j + 1) * 128],
                                         xs2[:, jj * 128:(jj + 1) * 128],
                                         biasr_sb[:], start=False, stop=True)
                    for j in range(np_):
                        t = t0 + 3 * h + j
                        nc.scalar.mul(outp[:, t, :],
                                      ps[:, j * 128:(j + 1) * 128],
                                      eeloop_sb[:, t:t + 1])

        # ---- Phase 2: window-major gather + aggregation ----
        roles = sched["roles"]
        tile_roles = sched["tile_roles"]
        up_groups = sched["up_groups"]
        up_role_off = sched["up_role_off"]
        bld_role_off = sched["bld_role_off"]
        groups = sched["groups"]
        segs = sched["segs"]
        acc_init = [False] * g.sh_tiles
        with tc.tile_pool(name="gp", bufs=10) as gp, \
             tc.tile_pool(name="mpu", bufs=3) as mpu, \
             tc.tile_pool(name="mpb", bufs=24) as mpb, \
             tc.tile_pool(name="ps2", bufs=8, space="PSUM") as ps2, \
             tc.tile_pool(name="op", bufs=3) as op:
            qn = 0
            for r in range(4):
                for gi in range(g.ng):
                    if (r, gi) not in segs:
                        continue
                    seg_first, seg_nch = segs[(r, gi)]
                    tiles = groups[gi]
                    G = gp.tile([128, seg_nch, 128], bf16, tag="G")
                    nc.gpsimd.dma_gather(
                        G[:], tw_d[r][:],
                        idx_sb[:, seg_first * 8:(seg_first + seg_nch) * 8],
                        seg_nch * 128, seg_nch * 128, 128,
                        single_packet=False, queue_num=qn % 4)
                    qn += 1
                    # upload / build role Ms for this (window, group)
                    seg_rids = [rid for t in tiles
                                for rid in tile_roles.get((t, r), [])]
                    Mu = None
                    ua = None
                    if gi in up_groups and seg_rids:
                        offs = [up_role_off[rid] for rid in seg_rids]
                        ua, ub = min(offs), max(offs) + 1
                        Mu = mpu.tile([128, ub - ua, 128], bf16, tag="Mu")
                        nc.sync.dma_start(Mu[:], mup_d[:, ua:ub, :])
                    role_m = {}
                    if gi not in up_groups:
                        for rid in seg_rids:
                            bo = bld_role_off[rid]
                            Mb = mpb.tile([128, 128], bf16, tag="Mb",
                                          name=f"mb{rid}")
                            nc.vector.tensor_scalar(
                                Mb[:], iota_sb[:],
                                dmodb_sb[:, bo:bo + 1], eeb_sb[:, bo:bo + 1],
                                Alu.is_equal, Alu.mult)
                            role_m[rid] = Mb
                    for t in tiles:
                        rids = tile_roles.get((t, r), [])
                        if not rids:
                            continue
                        pst = ps2.tile([128, g.hid], f32, tag="pst",
                                       name=f"pst{t}w{r}")
                        for i, rid in enumerate(rids):
                            k = roles[rid][0]
                            if gi in up_groups:
                                Ms = Mu[:, up_role_off[rid] - ua, :]
                            else:
                                Ms = role_m[rid][:]
                            nc.tensor.matmul(pst[:], Ms,
                                             G[:, k - seg_first, :],
                                             start=(i == 0),
                                             stop=(i == len(rids) - 1))
                        if not acc_init[t]:
                            nc.scalar.copy(acc[:, t, :], pst[:])
                            acc_init[t] = True
                        else:
                            nc.vector.tensor_tensor(acc[:, t, :], acc[:, t, :],
                                                    pst[:], Alu.add)
            # ---- final: out = relu(acc + out_partial), p-major write ----
            for gi in range(g.ng):
                tiles = groups[gi]
                obg = op.tile([128, len(tiles), g.hid], f32, tag="obg")
                for ti, t in enumerate(tiles):
                    if acc_init[t]:
                        nc.vector.tensor_tensor(obg[:, ti, :], acc[:, t, :],
                                                outp[:, t, :], Alu.add)
                    else:
                        nc.vector.tensor_copy(obg[:, ti, :], outp[:, t, :])
                    nc.vector.tensor_scalar(obg[:, ti, :], obg[:, ti, :], 0.0,
                                            None, Alu.max)
                nc.scalar.dma_start(
                    out_d[:, tiles[0]:tiles[0] + len(tiles), :],
                    obg[:, 0:len(tiles), :])
    nc.compile()
    return nc


def _in_maps(geo, shared, per_core):
    maps = []
    for c in range(geo.n_cores):
        m = dict(shared)
        m.update(per_core[c])
        maps.append(m)
    return maps


def _unshard(geo, res):
    """Assemble the full [N, HID] output from per-core p-major outputs."""
    outs = []
    for c in range(geo.n_cores):
        lo, hi = geo.core_dst_range(c)
        o = res.results[c]["out"]                      # [128, sh_tiles, hid]
        o = np.ascontiguousarray(o.transpose(1, 0, 2)).reshape(geo.sh, geo.hid)
        outs.append(o[:hi - lo])
    return np.concatenate(outs, axis=0).astype(np.float32)


def kernel(x, edge_index, W, att_src, att_dst, bias):
    from concourse.bass_utils import run_bass_kernel_spmd

    geo = Geo()
    shared, per_core, sched = _prep(geo, x, edge_index, W, att_src, att_dst, bias)
    nc = _build(geo, sched)
    in_maps = _in_maps(geo, shared, per_core)
    res = run_bass_kernel_spmd(nc, in_maps, core_ids=list(range(geo.n_cores)))
    return _unshard(geo, res)


def _emulate(geo, shared, per_core, sched):
    """Numpy emulation of the device program (for host-side validation)."""
    g = geo
    xT = shared["xt"].astype(np.float32)
    w = shared["w"].astype(np.float32)
    biasr = shared["biasr"].astype(np.float32)
    tab = (xT.T @ w).astype(BF16)                      # [ntab(node), hid]
    ptab = np.zeros_like(tab)
    ptab[np.asarray(g.perm_row(np.arange(g.ntab)))] = tab
    outs = []
    roles = sched["roles"]
    for c in range(g.n_cores):
        pc = per_core[c]
        lo, hi = g.core_dst_range(c)
        xto = pc["xto"].astype(np.float32)
        psum_own = (xto[:g.f_in].T @ w) + np.outer(xto[g.f_in], biasr[0])
        eelp = pc["eeloop"]
        outp = np.zeros((g.sh, g.hid), dtype=np.float32)
        for t in range(g.sh_tiles):
            sl = slice(t * 128, (t + 1) * 128)
            outp[sl] = psum_own[sl] * eelp[:, t][:, None]
        outp = outp.astype(BF16).astype(np.float32)
        idx16 = pc["idx"]
        nch = sched["nch"]
        G = np.zeros((nch * 128, g.hid), dtype=np.float32)
        for (r, gi), (seg_first, seg_nch) in sched["segs"].items():
            a, b = seg_first * 128, (seg_first + seg_nch) * 128
            flat = idx16[0:16, a // 16:b // 16].T.reshape(-1)
            G[a:b] = ptab[g.wrow0[r] + flat.astype(np.int64)]
        dmodb = pc["dmodb"]
        eeb = pc["eeb"]
        mupt = pc["mup"].transpose(1, 0, 2)
        out = np.zeros((g.sh, g.hid), dtype=np.float32)
        for t in range(g.sh_tiles):
            accv = np.zeros((128, g.hid), dtype=np.float32)
            first = True
            for r in range(4):
                pacc = np.zeros((128, g.hid), dtype=np.float32)
                rids = sched["tile_roles"].get((t, r), [])
                if not rids:
                    continue
                for rid in rids:
                    k, _, gi, _ = roles[rid]
                    Gk = G[k * 128:(k + 1) * 128]
                    if gi in sched["up_groups"]:
                        M = mupt[sched["up_role_off"][rid]].astype(np.float32)
                    else:
                        bo = sched["bld_role_off"][rid]
                        M = np.zeros((128, 128), dtype=np.float32)
                        M[np.arange(128), dmodb[:, bo].astype(np.int32)] = \
                            eeb[:, bo].astype(BF16).astype(np.float32)
                    pacc += M.T @ Gk.astype(BF16).astype(np.float32)
                # bf16 accumulator
                accv = (accv + pacc).astype(BF16).astype(np.float32) \
                    if not first else pacc.astype(BF16).astype(np.float32)
                first = False
            out[t * 128:(t + 1) * 128] = np.maximum(accv + outp[t * 128:(t + 1) * 128], 0.0)
        outs.append(out[:hi - lo])
    return np.concatenate(outs, axis=0)


if __name__ == "__main__":
    rng = np.random.RandomState(0)
    geo = Geo(n_nodes=8192, sh_tiles=8, group_tiles=3)
    x = rng.randn(8192, 256).astype(np.float32)
    ei = rng.randint(0, 8192, (2, 65536)).astype(np.int64)
    W = (rng.randn(256, 128) / 16).astype(np.float32)
    a1 = (rng.randn(128) / 11.3).astype(np.float32)
    a2 = (rng.randn(128) / 11.3).astype(np.float32)
    b = (rng.randn(128) * 0.1).astype(np.float32)
    sh, pc, sc = _prep(geo, x, ei, W, a1, a2, b)
    print("nch:", sc["nch"], "nslot:", sc["nslot"], "nroles:", sc["nroles"],
          "nup:", sc["nup"], "nbld:", sc["nbld"])

    def ref(x, ei, W, a1, a2, b):
        N = x.shape[0]
        xw = x @ W
        loops = np.arange(N)
        src = np.concatenate([ei[0], loops])
        dst = np.concatenate([ei[1], loops])
        a_s = xw @ a1
        a_d = xw @ a2
        e = a_s[src] + a_d[dst]
        e = np.where(e > 0, e, 0.2 * e)
        ee = np.exp(e)
        denom = np.zeros(N)
        np.add.at(denom, dst, ee)
        coef = ee / denom[dst]
        out = np.zeros((N, 128))
        np.add.at(out, dst, xw[src] * coef[:, None])
        return np.maximum(out + b, 0)

    exp = ref(x, ei, W, a1, a2, b)
    act = _emulate(geo, sh, pc, sc)
    rel = np.linalg.norm(act - exp) / np.linalg.norm(exp)
    print("emulation rel err:", rel)


# revision 10
# speedup vs baseline: 1.4532x; 1.4532x over previous
"""GAT (graph attention) kernel for Trainium2, 8-core SPMD.

Per core (dst-sharded), v4:
  The source-node table (xw = x @ W, 256B bf16 rows) is stored in
  BLOCK-MAJOR permuted order: 6-tile blocks, rows (block, p, tile) - so
  each phase-1 block write is one contiguous ~1.5KB descriptor per
  partition, AND each of the 4 int16 gather windows is a contiguous
  range of blocks held in its OWN dram tensor.  Gathers for window w
  therefore depend only on window w's table writes: the phase-2 gather
  stream starts ~75us in (after the first window is built), overlapping
  the rest of phase 1.  The gather stream is the kernel's critical
  resource (~2.7ns/row, 4 SWDGE queues); everything else hides under it.

  Phase 1b: own-shard pass computes out_partial[d] = ee_loop[d]*xw[d]+bias
  into SBUF bf16 (rank-1 bias matmul + ACT per-partition-scale copy).

  Phase 2 runs WINDOW-MAJOR: for each window w, for each group of `gsz`
  dst tiles: one dma_gather segment (per-tile slot ranges packed
  contiguously - only the segment rounds to 128 slots; boundary chunks
  get one routing matmul per tile, a "role").  A one-hot-times-ee matrix
  M[slot, dst] per role (DVE is_equal+mult build, or HBM upload for
  UPLOAD_NUM/UPLOAD_DEN of groups - uploads ride the idle HWDGE queues)
  routes edges: psum_t += M^T @ G; after each (tile, window):
  acc_t (+)= psum (bf16 SBUF accumulator).  Final: out = relu(acc +
  out_partial), written p-major (host un-permutes).

Host precomputes per-edge normalized attention coefficients
(two matvecs + O(E) scalar math, ~0.2% of total FLOPs) and the edge->slot
schedule; the feature matmul, the edge gather, aggregation matmuls and the
nonlinearity run on device.  Softmax max-subtraction is unnecessary:
a_s, a_d ~ N(0,1) so logits stay O(10) and exp() is safe in fp32.
Padding slots get ee = 0 so they contribute nothing.
"""

import numpy as np
import ml_dtypes

BF16 = ml_dtypes.bfloat16

# problem constants (nn_GAT_43593918054566)
N_NODES = 100000
F_IN = 256
HID = 128
NEG_SLOPE = 0.2
N_CORES = 8
UPLOAD_NUM, UPLOAD_DEN = 1, 1   # upload M for NUM of every DEN groups
BT = 6                          # table tiles per phase-1 block


class Geo:
    """Geometry/schedule shared by host prep and kernel builder."""

    def __init__(self, n_nodes=N_NODES, f_in=F_IN, hid=HID, n_cores=N_CORES,
                 sh_tiles=98, group_tiles=5):
        self.n = n_nodes
        self.f_in = f_in
        self.hid = hid
        self.n_cores = n_cores
        self.ntiles_tab = -(-n_nodes // 128)          # node tiles in table
        self.ntab = self.ntiles_tab * 128             # padded table rows
        self.sh_tiles = sh_tiles                      # dst tiles per core
        self.sh = sh_tiles * 128                      # dst shard stride
        assert self.sh * (n_cores - 1) < n_nodes <= self.sh * n_cores
        # phase-1 blocks of BT tiles; 4 gather windows = block ranges
        self.nblk = -(-self.ntiles_tab // BT)
        self.blk_nt = [min(BT, self.ntiles_tab - b * BT)
                       for b in range(self.nblk)]
        self.blk_row0 = np.concatenate(
            [[0], np.cumsum([nt * 128 for nt in self.blk_nt])]).astype(np.int64)
        nw = 4
        bw = -(-self.nblk // nw)
        self.wblk = [min(i * bw, self.nblk) for i in range(nw + 1)]
        self.wrow0 = [int(self.blk_row0[self.wblk[i]]) for i in range(nw + 1)]
        for i in range(nw):
            assert self.wrow0[i + 1] - self.wrow0[i] <= 32768
        self.gsz = group_tiles                        # dst tiles per group
        self.ng = -(-sh_tiles // group_tiles)

    def core_dst_range(self, c):
        lo = self.sh * c
        hi = min(lo + self.sh, self.n)
        return lo, hi

    def perm_row(self, n):
        """HBM table row of node n (block-major permuted layout)."""
        n = np.asarray(n)
        t = n >> 7
        p = n & 127
        b = t // BT
        a = t - b * BT
        nt = np.minimum(BT, self.ntiles_tab - b * BT)
        return self.blk_row0[b] + p * nt + a


def _prep(geo, x, edge_index, W, att_src, att_dst, bias):
    """Host preprocessing: edge partitioning + per-core input arrays."""
    g = geo
    x = np.asarray(x, dtype=np.float32)
    W = np.asarray(W, dtype=np.float32)
    esrc = np.asarray(edge_index[0], dtype=np.int64)
    edst = np.asarray(edge_index[1], dtype=np.int64)

    # per-edge normalized attention (host: 2 matvecs + O(E) scalar math)
    a_s = x @ (W @ np.asarray(att_src, np.float32))
    a_d = x @ (W @ np.asarray(att_dst, np.float32))

    def ee_of(s, d):
        e = a_s[s] + a_d[d]
        e = np.where(e > 0, e, NEG_SLOPE * e)
        return np.exp(e).astype(np.float32)

    ee_reg_all = ee_of(esrc, edst)
    ee_loop = ee_of(np.arange(g.n), np.arange(g.n))   # self loops
    denom = ee_loop.astype(np.float64).copy()
    np.add.at(denom, edst, ee_reg_all.astype(np.float64))
    ee_reg_all = (ee_reg_all / denom[edst]).astype(np.float32)
    ee_loop = (ee_loop / denom).astype(np.float32)

    # permuted gather rows + window of every edge source
    rperm_all = g.perm_row(esrc)
    wbs = np.asarray(g.wrow0[1:], dtype=np.int64)

    cores = []
    for c in range(g.n_cores):
        lo, hi = g.core_dst_range(c)
        m = (edst >= lo) & (edst < hi)
        d_c = edst[m] - lo
        t_c = d_c >> 7
        rp_c = rperm_all[m]
        r_c = np.searchsorted(wbs, rp_c, side="right")
        cores.append((rp_c, d_c, t_c, r_c, ee_reg_all[m]))

    # per-(tile, window) slot quota: max edge count over cores (NOT rounded)
    counts = np.zeros((g.n_cores, g.sh_tiles, 4), dtype=np.int64)
    for c, (_, _, t_c, r_c, _) in enumerate(cores):
        np.add.at(counts[c], (t_c, r_c), 1)
    Q = counts.max(axis=0)  # [T, 4]

    # slot layout: segment = (window, group); tiles packed contiguously,
    # segment rounded to 128.  Emission order is window-major: (r, gi).
    tile_slot0 = np.zeros((g.sh_tiles, 4), dtype=np.int64)
    segs = {}              # (r, gi) -> (first_chunk, n_chunks)
    roles = []             # (chunk, tile, gi, r) in emission order
    tile_roles = {}        # (t, r) -> [role_id...]
    groups = [list(range(gi * g.gsz, min((gi + 1) * g.gsz, g.sh_tiles)))
              for gi in range(g.ng)]
    off = 0
    for r in range(4):
        for gi in range(g.ng):
            tiles = groups[gi]
            seg_first_ch = off // 128
            so = off
            for t in tiles:
                tile_slot0[t, r] = so
                so += int(Q[t, r])
            so = -(-so // 128) * 128
            nchk = (so - off) // 128
            if nchk > 0:
                segs[(r, gi)] = (seg_first_ch, nchk)
            for t in tiles:
                a, b = tile_slot0[t, r], tile_slot0[t, r] + Q[t, r]
                if b == a:
                    continue
                for k in range(int(a // 128), int(-(-b // 128))):
                    rid = len(roles)
                    roles.append((k, t, gi, r))
                    tile_roles.setdefault((t, r), []).append(rid)
            off = so
    nch = off // 128
    nslot = off
    nroles = len(roles)

    up_groups = set(gi for gi in range(g.ng)
                    if (gi * UPLOAD_NUM) % UPLOAD_DEN < UPLOAD_NUM)
    up_role_off = {}
    bld_role_off = {}
    nup = nbld = 0
    for rid, (k, t, gi, r) in enumerate(roles):
        if gi in up_groups:
            up_role_off[rid] = nup
            nup += 1
        else:
            bld_role_off[rid] = nbld
            nbld += 1
    nup_a = max(nup, 1)
    nbld_a = max(nbld, 1)

    # slot -> tile map (core-uniform)
    slot_tile = np.full(nslot, -1, dtype=np.int64)
    for t in range(g.sh_tiles):
        for r in range(4):
            a = tile_slot0[t, r]
            slot_tile[a:a + Q[t, r]] = t
    rid_map = np.full(nch * g.sh_tiles, -1, dtype=np.int64)
    for rid, (k, t, gi, r) in enumerate(roles):
        rid_map[k * g.sh_tiles + t] = rid

    per_core = []
    for c, (rp_c, d_c, t_c, r_c, ee_c) in enumerate(cores):
        lo, hi = g.core_dst_range(c)
        idx_flat = np.zeros(nslot, dtype=np.int16)
        dmod = np.zeros(nslot, dtype=np.int32)
        eesl = np.zeros(nslot, dtype=np.float32)
        order = np.lexsort((t_c, r_c))
        rp_o, d_o, t_o, r_o = rp_c[order], d_c[order], t_c[order], r_c[order]
        ee_o = ee_c[order]
        run_id = r_o * g.sh_tiles + t_o
        run_starts = np.searchsorted(run_id, np.arange(4 * g.sh_tiles))
        rank = np.arange(len(rp_o)) - run_starts[run_id]
        slot = tile_slot0[t_o, r_o] + rank
        idx_flat[slot] = (rp_o - np.asarray(g.wrow0, dtype=np.int64)[r_o]).astype(np.int16)
        dmod[slot] = (d_o & 127).astype(np.int32)
        eesl[slot] = ee_o

        # wrap gather idx per segment: pos i -> [16k + i%16, i//16]
        idx16 = np.zeros((128, nslot // 16), dtype=np.int16)
        for (r, gi), (seg_first, seg_nch) in segs.items():
            a, b = seg_first * 128, (seg_first + seg_nch) * 128
            wrapped = idx_flat[a:b].reshape(-1, 16).T
            cols = slice(a // 16, b // 16)
            for k in range(8):
                idx16[16 * k:16 * k + 16, cols] = wrapped
        # per-role dmod/ee columns (ee zero outside the role's tile range)
        dmodb = np.zeros((128, nbld_a), dtype=np.float32)
        eeb = np.zeros((128, nbld_a), dtype=np.float32)
        mh = np.zeros((nup_a, 128, 128), dtype=BF16)
        svalid = slot_tile >= 0
        sv = np.nonzero(svalid)[0]
        rid_of = rid_map[(sv // 128) * g.sh_tiles + slot_tile[sv]]
        up_m = np.asarray([gi2 in up_groups for (_, _, gi2, _) in roles])
        moff = np.asarray([up_role_off.get(rid, -1) for rid in range(nroles)])
        boff = np.asarray([bld_role_off.get(rid, -1) for rid in range(nroles)])
        u = up_m[rid_of]
        mh[moff[rid_of[u]], sv[u] % 128, dmod[sv[u]]] = eesl[sv[u]].astype(BF16)
        bsel = rid_of[~u]
        dmodb[sv[~u] % 128, boff[bsel]] = dmod[sv[~u]].astype(np.float32)
        eeb[sv[~u] % 128, boff[bsel]] = eesl[sv[~u]]
        mh = np.ascontiguousarray(mh.transpose(1, 0, 2))  # [128, nup, 128]
        # own-shard x (transposed, zero-padded) + rank-1 bias row 1/ee_loop
        nd = hi - lo
        xto = np.zeros((g.f_in + 1, g.sh), dtype=BF16)
        xto[:g.f_in, :nd] = x[lo:hi].T.astype(BF16)
        xto[g.f_in, :nd] = (1.0 / ee_loop[lo:hi]).astype(BF16)
        el = np.zeros(g.sh, dtype=np.float32)
        el[:nd] = ee_loop[lo:hi]
        eelp = np.ascontiguousarray(el.reshape(g.sh_tiles, 128).T)
        per_core.append({"idx": idx16, "mup": mh, "xto": xto,
                         "dmodb": dmodb, "eeb": eeb, "eeloop": eelp})

    # x transposed, node-major: the phase-1 write AP performs the
    # block-major row permutation (s[p, a] -> row lrow + p*nt + a)
    xT = np.zeros((g.f_in, g.ntab), dtype=BF16)
    xT[:, :g.n] = x.T.astype(BF16)
    wbf = np.ascontiguousarray(W.astype(BF16))
    biasr = np.asarray(bias, np.float32).astype(BF16)[None, :]
    iota128 = np.ascontiguousarray(
        np.tile(np.arange(128, dtype=np.float32).astype(BF16), (128, 1)))

    shared = {"xt": xT, "w": wbf, "biasr": biasr, "iota128": iota128}
    sched = {"nch": nch, "nslot": nslot, "nroles": nroles,
             "nup": nup_a, "nbld": nbld_a, "segs": segs, "roles": roles,
             "tile_roles": tile_roles, "groups": groups,
             "up_groups": up_groups, "up_role_off": up_role_off,
             "bld_role_off": bld_role_off}
    return shared, per_core, sched


def _build(geo, sched):
    """Build the (core-uniform) Bass program."""
    import concourse.bacc as bacc
    import concourse.mybir as mybir
    from concourse import tile
    from contextlib import ExitStack

    g = geo
    nch, nslot = sched["nch"], sched["nslot"]
    f32, bf16 = mybir.dt.float32, mybir.dt.bfloat16
    i16 = mybir.dt.int16
    Alu = mybir.AluOpType

    nc = bacc.Bacc("TRN2", target_bir_lowering=False, debug=False,
                   num_devices=g.n_cores, num_swdge_queues=4)

    xt_d = nc.dram_tensor("xt", [g.f_in, g.ntab], bf16, kind="ExternalInput")
    xto_d = nc.dram_tensor("xto", [g.f_in + 1, g.sh], bf16, kind="ExternalInput")
    w_d = nc.dram_tensor("w", [g.f_in, g.hid], bf16, kind="ExternalInput")
    biasr_d = nc.dram_tensor("biasr", [1, g.hid], bf16, kind="ExternalInput")
    idx_d = nc.dram_tensor("idx", [128, nslot // 16], i16, kind="ExternalInput")
    mup_d = nc.dram_tensor("mup", [128, sched["nup"], 128], bf16,
                           kind="ExternalInput")
    iota_d = nc.dram_tensor("iota128", [128, 128], bf16, kind="ExternalInput")
    dmodb_d = nc.dram_tensor("dmodb", [128, sched["nbld"]], f32,
                             kind="ExternalInput")
    eeb_d = nc.dram_tensor("eeb", [128, sched["nbld"]], f32,
                           kind="ExternalInput")
    eeloop_d = nc.dram_tensor("eeloop", [128, g.sh_tiles], f32,
                              kind="ExternalInput")
    out_d = nc.dram_tensor("out", [128, g.sh_tiles, g.hid], f32,
                           kind="ExternalOutput")
    # one table tensor per gather window -> exact write->gather deps
    tw_d = [nc.dram_tensor(f"tw{w}", [g.wrow0[w + 1] - g.wrow0[w], 128],
                           bf16, kind="Internal") for w in range(4)]

    with tile.TileContext(nc) as tc, ExitStack() as ctx:
        const = ctx.enter_context(tc.tile_pool(name="const", bufs=1))
        w0 = const.tile([128, g.hid], bf16)
        w1 = const.tile([128, g.hid], bf16)
        nc.sync.dma_start(w0[:], w_d[0:128, :])
        nc.sync.dma_start(w1[:], w_d[128:256, :])
        biasr_sb = const.tile([1, g.hid], bf16)
        nc.sync.dma_start(biasr_sb[:], biasr_d[:])
        idx_sb = const.tile([128, nslot // 16], i16)
        nc.sync.dma_start(idx_sb[:], idx_d[:])
        iota_sb = const.tile([128, 128], bf16)
        nc.sync.dma_start(iota_sb[:], iota_d[:])
        dmodb_sb = const.tile([128, sched["nbld"]], f32)
        nc.sync.dma_start(dmodb_sb[:], dmodb_d[:])
        eeb_sb = const.tile([128, sched["nbld"]], f32)
        nc.sync.dma_start(eeb_sb[:], eeb_d[:])
        eeloop_sb = const.tile([128, g.sh_tiles], f32)
        nc.sync.dma_start(eeloop_sb[:], eeloop_d[:])
        outp = const.tile([128, g.sh_tiles, g.hid], bf16)
        acc = const.tile([128, g.sh_tiles, g.hid], bf16)

        xp = ctx.enter_context(tc.tile_pool(name="xp", bufs=4))
        stag = ctx.enter_context(tc.tile_pool(name="stag", bufs=3))
        ps1 = ctx.enter_context(tc.tile_pool(name="ps1", bufs=4, space="PSUM"))
        gp = ctx.enter_context(tc.tile_pool(name="gp", bufs=10))
        mpu = ctx.enter_context(tc.tile_pool(name="mpu", bufs=3))
        mpb = ctx.enter_context(tc.tile_pool(name="mpb", bufs=24))
        ps2 = ctx.enter_context(tc.tile_pool(name="ps2", bufs=4, space="PSUM"))
        op = ctx.enter_context(tc.tile_pool(name="op", bufs=3))

        # ---- Phase 1a: node table, block-major windows ----
        if True:
            for b in range(g.nblk):
                t0 = BT * b
                nt = g.blk_nt[b]
                w = next(i for i in range(4)
                         if g.wblk[i] <= b < g.wblk[i + 1])
                lrow = int(g.blk_row0[b]) - g.wrow0[w]
                xs0 = xp.tile([128, nt * 128], bf16, tag="xs0")
                xs1 = xp.tile([128, nt * 128], bf16, tag="xs1")
                c0 = int(g.blk_row0[b])
                nc.sync.dma_start(xs0[:], xt_d[0:128, c0:c0 + nt * 128])
                nc.sync.dma_start(xs1[:], xt_d[128:256, c0:c0 + nt * 128])
                s = stag.tile([128, nt, 128], bf16, tag="s")
                for h in range(-(-nt // 3)):
                    np_ = min(3, nt - 3 * h)
                    ps = ps1.tile([128, np_ * 128], f32, tag="ps1t")
                    for j in range(np_):
                        jj = 3 * h + j
                        nc.tensor.matmul(ps[:, j * 128:(j + 1) * 128],
                                         xs0[:, jj * 128:(jj + 1) * 128],
                                         w0[:], start=True, stop=False)
                        nc.tensor.matmul(ps[:, j * 128:(j + 1) * 128],
                                         xs1[:, jj * 128:(jj + 1) * 128],
                                         w1[:], start=False, stop=True)
                    nc.scalar.copy(s[:, 3 * h:3 * h + np_, :],
                                   ps[:].rearrange("p (a b) -> p a b", b=128))
                nc.scalar.dma_start(
                    tw_d[w][lrow:lrow + nt * 128, :].rearrange(
                        "(p a) e -> p a e", p=128),
                    s[:])

            # ---- Phase 1b: own shard -> out_partial ----
            for b in range(-(-g.sh_tiles // 6)):
                t0 = 6 * b
                nt = min(6, g.sh_tiles - t0)
                xs0 = xp.tile([128, nt * 128], bf16, tag="xs0")
                xs1 = xp.tile([128, nt * 128], bf16, tag="xs1")
                xs2 = xp.tile([1, nt * 128], bf16, tag="xs2")
                nc.sync.dma_start(xs0[:], xto_d[0:128, t0 * 128:(t0 + nt) * 128])
                nc.sync.dma_start(xs1[:], xto_d[128:256, t0 * 128:(t0 + nt) * 128])
                nc.sync.dma_start(xs2[:], xto_d[256:257, t0 * 128:(t0 + nt) * 128])
                for h in range(-(-nt // 3)):
                    np_ = min(3, nt - 3 * h)
                    ps = ps1.tile([128, np_ * 128], f32, tag="ps1t")
                    for j in range(np_):
                        jj = 3 * h + j
                        nc.tensor.matmul(ps[:, j * 128:(j + 1) * 128],
                                         xs0[:, jj * 128:(jj + 1) * 128],
                                         w0[:], start=True, stop=False)
                        nc.tensor.matmul(ps[:, j * 128:(j + 1) * 128],
                                         xs1[:, jj * 128:(jj + 1) * 128],
                                         w1[:], start=False, stop=False)
                        nc.tensor.matmul(ps[:, j * 128:(j + 1) * 128],
                                         xs2[:, jj * 128:(jj + 1) * 128],
                                         biasr_sb[:], start=False, stop=True)
                    for j in range(np_):
                        t = t0 + 3 * h + j
                        nc.scalar.mul(outp[:, t, :],
                                      ps[:, j * 128:(j + 1) * 128],
                                      eeloop_sb[:, t:t + 1])

        # ---- Phase 2: window-major gather + aggregation ----
        roles = sched["roles"]
        tile_roles = sched["tile_roles"]
        up_groups = sched["up_groups"]
        up_role_off = sched["up_role_off"]
        bld_role_off = sched["bld_role_off"]
        groups = sched["groups"]
        segs = sched["segs"]
        acc_init = [False] * g.sh_tiles
        if True:
            qn = 0
            for r in range(4):
                for gi in range(g.ng):
                    if (r, gi) not in segs:
                        continue
                    seg_first, seg_nch = segs[(r, gi)]
                    tiles = groups[gi]
                    G = gp.tile([128, seg_nch, 128], bf16, tag="G")
                    nc.gpsimd.dma_gather(
                        G[:], tw_d[r][:],
                        idx_sb[:, seg_first * 8:(seg_first + seg_nch) * 8],
                        seg_nch * 128, seg_nch * 128, 128,
                        single_packet=False, queue_num=qn % 4)
                    qn += 1
                    # upload / build role Ms for this (window, group)
                    seg_rids = [rid for t in tiles
                                for rid in tile_roles.get((t, r), [])]
                    Mu = None
                    ua = None
                    if gi in up_groups and seg_rids:
                        offs = [up_role_off[rid] for rid in seg_rids]
                        ua, ub = min(offs), max(offs) + 1
                        Mu = mpu.tile([128, ub - ua, 128], bf16, tag="Mu")
                        nc.sync.dma_start(Mu[:], mup_d[:, ua:ub, :])
                    role_m = {}
                    if gi not in up_groups:
                        for rid in seg_rids:
                            bo = bld_role_off[rid]
                            Mb = mpb.tile([128, 128], bf16, tag="Mb",
                                          name=f"mb{rid}")
                            nc.vector.tensor_scalar(
                                Mb[:], iota_sb[:],
                                dmodb_sb[:, bo:bo + 1], eeb_sb[:, bo:bo + 1],
                                Alu.is_equal, Alu.mult)
                            role_m[rid] = Mb
                    for t in tiles:
                        rids = tile_roles.get((t, r), [])
                        if not rids:
                            continue
                        pst = ps2.tile([128, g.hid], f32, tag="pst",
                                       name=f"pst{t}w{r}")
                        for i, rid in enumerate(rids):
                            k = roles[rid][0]
                            if gi in up_groups:
                                Ms = Mu[:, up_role_off[rid] - ua, :]
                            else:
                                Ms = role_m[rid][:]
                            nc.tensor.matmul(pst[:], Ms,
                                             G[:, k - seg_first, :],
                                             start=(i == 0),
                                             stop=(i == len(rids) - 1))
                        if not acc_init[t]:
                            nc.scalar.copy(acc[:, t, :], pst[:])
                            acc_init[t] = True
                        else:
                            nc.vector.tensor_tensor(acc[:, t, :], acc[:, t, :],
                                                    pst[:], Alu.add)
            # ---- final: out = relu(acc + out_partial), p-major write ----
            for gi in range(g.ng):
                tiles = groups[gi]
                obg = op.tile([128, len(tiles), g.hid], f32, tag="obg")
                for ti, t in enumerate(tiles):
                    if acc_init[t]:
                        nc.vector.tensor_tensor(obg[:, ti, :], acc[:, t, :],
                                                outp[:, t, :], Alu.add)
                    else:
                        nc.vector.tensor_copy(obg[:, ti, :], outp[:, t, :])
                    nc.vector.tensor_scalar(obg[:, ti, :], obg[:, ti, :], 0.0,
                                            None, Alu.max)
                nc.scalar.dma_start(
                    out_d[:, tiles[0]:tiles[0] + len(tiles), :],
                    obg[:, 0:len(tiles), :])
    nc.compile()
    return nc


def _in_maps(geo, shared, per_core):
    maps = []
    for c in range(geo.n_cores):
        m = dict(shared)
        m.update(per_core[c])
        maps.append(m)
    return maps


def _unshard(geo, res):
    """Assemble the full [N, HID] output from per-core p-major outputs."""
    outs = []
    for c in range(geo.n_cores):
        lo, hi = geo.core_dst_range(c)
        o = res.results[c]["out"]                      # [128, sh_tiles, hid]
        o = np.ascontiguousarray(o.transpose(1, 0, 2)).reshape(geo.sh, geo.hid)
        outs.append(o[:hi - lo])
    return np.concatenate(outs, axis=0).astype(np.float32)


def kernel(x, edge_index, W, att_src, att_dst, bias):
    from concourse.bass_utils import run_bass_kernel_spmd

    geo = Geo()
    shared, per_core, sched = _prep(geo, x, edge_index, W, att_src, att_dst, bias)
    nc = _build(geo, sched)
    in_maps = _in_maps(geo, shared, per_core)
    res = run_bass_kernel_spmd(nc, in_maps, core_ids=list(range(geo.n_cores)))
    return _unshard(geo, res)


def _emulate(geo, shared, per_core, sched):
    """Numpy emulation of the device program (for host-side validation)."""
    g = geo
    xT = shared["xt"].astype(np.float32)
    w = shared["w"].astype(np.float32)
    biasr = shared["biasr"].astype(np.float32)
    tab = (xT.T @ w).astype(BF16)                      # [ntab(node), hid]
    ptab = np.zeros_like(tab)
    ptab[np.asarray(g.perm_row(np.arange(g.ntab)))] = tab
    outs = []
    roles = sched["roles"]
    for c in range(g.n_cores):
        pc = per_core[c]
        lo, hi = g.core_dst_range(c)
        xto = pc["xto"].astype(np.float32)
        psum_own = (xto[:g.f_in].T @ w) + np.outer(xto[g.f_in], biasr[0])
        eelp = pc["eeloop"]
        outp = np.zeros((g.sh, g.hid), dtype=np.float32)
        for t in range(g.sh_tiles):
            sl = slice(t * 128, (t + 1) * 128)
            outp[sl] = psum_own[sl] * eelp[:, t][:, None]
        outp = outp.astype(BF16).astype(np.float32)
        idx16 = pc["idx"]
        nch = sched["nch"]
        G = np.zeros((nch * 128, g.hid), dtype=np.float32)
        for (r, gi), (seg_first, seg_nch) in sched["segs"].items():
            a, b = seg_first * 128, (seg_first + seg_nch) * 128
            flat = idx16[0:16, a // 16:b // 16].T.reshape(-1)
            G[a:b] = ptab[g.wrow0[r] + flat.astype(np.int64)]
        dmodb = pc["dmodb"]
        eeb = pc["eeb"]
        mupt = pc["mup"].transpose(1, 0, 2)
        out = np.zeros((g.sh, g.hid), dtype=np.float32)
        for t in range(g.sh_tiles):
            accv = np.zeros((128, g.hid), dtype=np.float32)
            first = True
            for r in range(4):
                pacc = np.zeros((128, g.hid), dtype=np.float32)
                rids = sched["tile_roles"].get((t, r), [])
                if not rids:
                    continue
                for rid in rids:
                    k, _, gi, _ = roles[rid]
                    Gk = G[k * 128:(k + 1) * 128]
                    if gi in sched["up_groups"]:
                        M = mupt[sched["up_role_off"][rid]].astype(np.float32)
                    else:
                        bo = sched["bld_role_off"][rid]
                        M = np.zeros((128, 128), dtype=np.float32)
                        M[np.arange(128), dmodb[:, bo].astype(np.int32)] = \
                            eeb[:, bo].astype(BF16).astype(np.float32)
                    pacc += M.T @ Gk.astype(BF16).astype(np.float32)
                # bf16 accumulator
                accv = (accv + pacc).astype(BF16).astype(np.float32) \
                    if not first else pacc.astype(BF16).astype(np.float32)
                first = False
            out[t * 128:(t + 1) * 128] = np.maximum(accv + outp[t * 128:(t + 1) * 128], 0.0)
        outs.append(out[:hi - lo])
    return np.concatenate(outs, axis=0)


if __name__ == "__main__":
    rng = np.random.RandomState(0)
    geo = Geo(n_nodes=8192, sh_tiles=8, group_tiles=3)
    x = rng.randn(8192, 256).astype(np.float32)
    ei = rng.randint(0, 8192, (2, 65536)).astype(np.int64)
    W = (rng.randn(256, 128) / 16).astype(np.float32)
    a1 = (rng.randn(128) / 11.3).astype(np.float32)
    a2 = (rng.randn(128) / 11.3).astype(np.float32)
    b = (rng.randn(128) * 0.1).astype(np.float32)
    sh, pc, sc = _prep(geo, x, ei, W, a1, a2, b)
    print("nch:", sc["nch"], "nslot:", sc["nslot"], "nroles:", sc["nroles"],
          "nup:", sc["nup"], "nbld:", sc["nbld"])

    def ref(x, ei, W, a1, a2, b):
        N = x.shape[0]
        xw = x @ W
        loops = np.arange(N)
        src = np.concatenate([ei[0], loops])
        dst = np.concatenate([ei[1], loops])
        a_s = xw @ a1
        a_d = xw @ a2
        e = a_s[src] + a_d[dst]
        e = np.where(e > 0, e, 0.2 * e)
        ee = np.exp(e)
        denom = np.zeros(N)
        np.add.at(denom, dst, ee)
        coef = ee / denom[dst]
        out = np.zeros((N, 128))
        np.add.at(out, dst, xw[src] * coef[:, None])
        return np.maximum(out + b, 0)

    exp = ref(x, ei, W, a1, a2, b)
    act = _emulate(geo, sh, pc, sc)
    rel = np.linalg.norm(act - exp) / np.linalg.norm(exp)
    print("emulation rel err:", rel)
